# revision 9
# baseline (speedup 1.0000x reference)
"""Trainium2 Bass kernel for AdaptiveLRLinearWithChannel (moe_routing).

Math: out[n] = x[n] @ reshape(U[idx[n]] @ V, [IN, OUT]) + bias[idx[n]]
  x: [256, 1024, 256] f32, U: [512, 60], V: [60, 65536], bias: [512, 1, 256]

Strategy (8 NeuronCores, data/expert parallel over the selected-channel dim):
  - Host (sharding/layout layer): shard the 256 selected channels 32 per
    core; gather the per-channel weights W = (U @ V)[idx] and bias rows by
    indices; lay W out as [i%128, channel, i//128, o] and x as
    [channel, IN, B] so the contraction dim (IN) lands on SBUF partitions.
    The low-rank weight synthesis is cheap preprocessing (2 GFLOP, ~6% of
    total FLOPs); the 34.4 GFLOP batched einsum runs on the device, which
    is what the kernel is memory-bound on (x in + out out = 67MB/core).
  - Device: per channel, per 128-row batch chunk: two accumulating fp32r
    matmuls (K=128 each) into PSUM, DVE bias-add into an SBUF staging
    tile, batched 512KB DMA to the output.
"""

import sys

for _p in ("/opt/trn_rl_repo",):
    if _p not in sys.path:
        sys.path.append(_p)

import numpy as np

from concourse import bacc
import concourse.mybir as mybir
import concourse.bass_utils as bass_utils
from concourse.tile import TileContext

N_CORES = 8
N_SEL = 256
B = 1024
IN = 256
OUT = 256
RANK = 60

N_LOC = N_SEL // N_CORES          # 32 channels per core
K_CH = IN // 128                  # 2 i-chunks of 128
B_CH = B // 128                   # 8 batch chunks of 128
OG = 4                            # batch chunks per output staging group

F32 = mybir.dt.float32
F32R = mybir.dt.float32r

_NC_CACHE = None


def _build():
    nc = bacc.Bacc()
    xt = nc.declare_dram_parameter("xt", [N_LOC, IN, B], F32, isOutput=False)
    w2d = nc.declare_dram_parameter("w2", [128, N_LOC, K_CH, OUT], F32, isOutput=False)
    bias = nc.declare_dram_parameter("bias", [N_LOC, OUT], F32, isOutput=False)
    out = nc.declare_dram_parameter("out", [N_LOC, B, OUT], F32, isOutput=True)

    W2_GRP = 8  # channels per W2-load chunk (lets channel-0 compute start early)

    with TileContext(nc) as tc:
        with (
            tc.tile_pool(name="const", bufs=1) as cpool,
            tc.tile_pool(name="xp", bufs=6) as xpool,
            tc.tile_pool(name="bp", bufs=2) as bpool,
            tc.tile_pool(name="op", bufs=4) as opool,
            tc.tile_pool(name="psm", bufs=6, space="PSUM") as psmp,
        ):
            # W2[p, c, k, o] = W[c, k*128+p, o]; rhs slices are W2[:, c, k, :]
            W2 = cpool.tile([128, N_LOC, K_CH, OUT], F32R)
            for c0 in range(0, N_LOC, W2_GRP):
                nc.sync.dma_start(
                    out=W2[:, c0 : c0 + W2_GRP, :, :],
                    in_=w2d[:, c0 : c0 + W2_GRP, :, :].bitcast(F32R),
                )
            # all 32 bias rows on partition 0; broadcast per channel via gpsimd
            brow = cpool.tile([1, N_LOC * OUT], F32)
            nc.sync.dma_start(
                out=brow[:], in_=bias[:].rearrange("c o -> (c o)").unsqueeze(0)
            )

            for c in range(N_LOC):
                xs = xpool.tile([128, K_CH, B], F32R)
                nc.sync.dma_start(
                    out=xs[:],
                    in_=xt[c].rearrange("(k p) b -> p k b", p=128).bitcast(F32R),
                )
                bb = bpool.tile([128, OUT], F32)
                nc.gpsimd.partition_broadcast(bb[:], brow[0:1, c * OUT : (c + 1) * OUT])
                for g in range(B_CH // OG):
                    # psum partition p of matmul j holds batch row g*512 + 4p + j,
                    # so each osb partition holds 4 consecutive output rows and
                    # the store below is 4KB-contiguous per partition.
                    osb = opool.tile([128, OG, OUT], F32)
                    for j in range(OG):
                        b0 = g * (OG * 128) + j
                        po = psmp.tile([128, OUT], F32)
                        nc.tensor.matmul(
                            po[:],
                            xs[:, 0, b0 : b0 + OG * 127 + 1 : OG],
                            W2[:, c, 0, :],
                            start=True,
                            stop=False,
                        )
                        nc.tensor.matmul(
                            po[:],
                            xs[:, 1, b0 : b0 + OG * 127 + 1 : OG],
                            W2[:, c, 1, :],
                            start=False,
                            stop=True,
                        )
                        nc.vector.tensor_add(osb[:, j, :], po[:], bb[:])
                    nc.scalar.dma_start(
                        out=out[c].rearrange("(g p j) o -> g p (j o)", p=128, j=OG)[g],
                        in_=osb[:].rearrange("p j o -> p (j o)"),
                    )
    nc.finalize()
    return nc


def _get_nc():
    global _NC_CACHE
    if _NC_CACHE is None:
        _NC_CACHE = _build()
    return _NC_CACHE


def make_in_maps(x, indices, weights_U, weights_V, bias):
    x = np.asarray(x, dtype=np.float32)
    idx = np.asarray(indices).astype(np.int64)
    u = np.asarray(weights_U, dtype=np.float32)
    v = np.asarray(weights_V, dtype=np.float32)
    b = np.asarray(bias, dtype=np.float32)

    # Per-channel weight gather + low-rank synthesis (preprocessing).
    w_sel = (u[idx] @ v).reshape(N_SEL, K_CH, 128, OUT)  # [n, k, p, o]

    in_maps = []
    for core in range(N_CORES):
        s = slice(core * N_LOC, (core + 1) * N_LOC)
        ii = idx[s]
        in_maps.append(
            {
                "xt": np.ascontiguousarray(x[s].transpose(0, 2, 1)),
                "w2": np.ascontiguousarray(w_sel[s].transpose(2, 0, 1, 3)),
                "bias": np.ascontiguousarray(b[ii, 0, :]),
            }
        )
    return in_maps


def kernel(x, indices, weights_U, weights_V, bias):
    in_maps = make_in_maps(x, indices, weights_U, weights_V, bias)
    nc = _get_nc()
    res = bass_utils.run_bass_kernel_spmd(nc, in_maps, core_ids=list(range(N_CORES)))
    return np.concatenate([res.results[i]["out"] for i in range(N_CORES)], axis=0)
